# revision 19
# baseline (speedup 1.0000x reference)
"""Multi-head attention (nn_MultiHeadAttention) Trainium2 Bass kernel.

Sharding: 8 cores = 4 batches x 2 head-groups (8 heads each).
Per core, heads are processed in pairs living on partition rows 0-63 /
64-127 so the K=64 S-matmuls alternate PE row-groups (concurrent sub-array
execution). The key mask folds into the exp bias; O and the softmax
denominator come from one M=65 matmul (ones column in vT) over the
unnormalized exp, so PE never waits on the normalize chain. The exp tiles,
vT, and the normalize factor are bf16: DVE runs the in-place normalize in
2x mode and the att DMA moves half the bytes. S scores and the k/q/o/w
tensors stay float32r. Host adds x + the two head-group partials.
"""
import sys
sys.path.insert(0, "/opt/trn_rl_repo")

import numpy as np

B, C, T, H = 4, 1024, 1024, 16
D = C // H            # 64 head dim
HPC = 8               # heads per core
NCORES = 8

_CACHE = {}


def _build_program():
    import concourse.bass as bass
    import concourse.mybir as mybir
    import concourse.tile as tile
    from concourse import bacc

    f32 = mybir.dt.float32
    f32r = mybir.dt.float32r
    bf16 = mybir.dt.bfloat16

    nc = bacc.Bacc("TRN2", target_bir_lowering=False, debug=False,
                   num_devices=NCORES)

    x_d = nc.dram_tensor("x", [C, T], f32r, kind="ExternalInput").ap()
    wk_d = nc.dram_tensor("wk", [C, 512], f32r, kind="ExternalInput").ap()
    wq_d = nc.dram_tensor("wq", [C, 512], f32r, kind="ExternalInput").ap()
    wv_d = nc.dram_tensor("wv", [C, 512], f32r, kind="ExternalInput").ap()
    wo_d = nc.dram_tensor("wo", [512, C], f32r, kind="ExternalInput").ap()
    kmask_d = nc.dram_tensor("kmask", [128, 8], f32, kind="ExternalInput").ap()
    qmask_d = nc.dram_tensor("qmask", [1, T], f32, kind="ExternalInput").ap()
    ones_d = nc.dram_tensor("ones", [128, 1], bf16, kind="ExternalInput").ap()

    att_d = nc.dram_tensor("att", [HPC, T, T], bf16, kind="ExternalOutput").ap()
    out_d = nc.dram_tensor("out", [C, T], f32r, kind="ExternalOutput").ap()

    Exp = mybir.ActivationFunctionType.Exp

    with tile.TileContext(nc) as tc:
        with (
            tc.tile_pool(name="persist", bufs=1) as persist,
            tc.tile_pool(name="out_sb", bufs=2) as out_pool,
        ):
            k_sb = persist.tile([128, 4, T], f32r)
            q_sb = persist.tile([128, 4, T], f32r)
            # per head: 64 v columns + a ones column => 65 wide (bf16)
            vT_sb = persist.tile([128, 8, 8 * 65], bf16)
            o_all = persist.tile([128, 4, T], f32r)
            kmask_sb = persist.tile([128, 8], f32)
            qmask_sb = persist.tile([1, T], f32)
            stage_sb = persist.tile([64, T], f32r)

            nc.scalar.dma_start(out=kmask_sb[:], in_=kmask_d[:])
            nc.gpsimd.dma_start(out=qmask_sb[:], in_=qmask_d[:])
            # ones columns of vT (one per head, column 65h+64): memset a ones
            # tile and fan it out with one strided DVE copy (a DMA here would
            # emit thousands of 2-byte descriptors)
            ones_sb = persist.tile([128, 1], bf16)
            nc.vector.memset(ones_sb[:], 1.0)
            ones_rep = bass.AP(
                tensor=ones_sb[:].tensor, offset=ones_sb[:].offset,
                ap=[list(p) for p in ones_sb[:].ap[:1]] + [[0, 8], [0, 8], [0, 1]])
            nc.vector.tensor_copy(
                vT_sb[:].rearrange("p m (h e) -> p m h e", e=65)[:, :, :, 64:65],
                ones_rep)

            # ---- projections ----
            with (
                tc.tile_pool(name="projin", bufs=1) as projin,
                tc.tile_pool(name="proj_ps", bufs=4, space="PSUM") as proj_ps,
            ):
                x_sb = projin.tile([128, 8, T], f32r)
                wk_sb = projin.tile([128, 8, 512], f32r)
                wq_sb = projin.tile([128, 8, 512], f32r)
                wv_sb = projin.tile([128, 8, 512], f32r)
                # one big DMA per tensor (contiguous dst = full ring rate),
                # spread across the three DGE queues
                nc.sync.dma_start(
                    out=x_sb[:], in_=x_d.rearrange("(a p) t -> p a t", p=128))
                nc.scalar.dma_start(
                    out=wk_sb[:], in_=wk_d.rearrange("(a p) m -> p a m", p=128))
                nc.gpsimd.dma_start(
                    out=wq_sb[:], in_=wq_d.rearrange("(a p) m -> p a m", p=128))
                nc.scalar.dma_start(
                    out=wv_sb[:], in_=wv_d.rearrange("(a p) m -> p a m", p=128))

                # k, q projections: out rows m-tile (128 rows = head pair)
                for m in range(4):
                    for w_sb, dst in ((wk_sb, k_sb), (wq_sb, q_sb)):
                        for n in range(2):
                            ps = proj_ps.tile([128, 512], f32, tag="pp")
                            for a in range(8):
                                nc.tensor.matmul(
                                    ps[:],
                                    w_sb[:, a, 128 * m:128 * m + 128],
                                    x_sb[:, a, 512 * n:512 * n + 512],
                                    start=(a == 0), stop=(a == 7),
                                )
                            nc.vector.tensor_copy(
                                dst[:, m, 512 * n:512 * n + 512], ps[:])
                # vT projection: out [t-tile, 512 cv] -> strided per-head 65 layout
                for m in range(8):
                    ps = proj_ps.tile([128, 512], f32, tag="pp")
                    for a in range(8):
                        nc.tensor.matmul(
                            ps[:],
                            x_sb[:, a, 128 * m:128 * m + 128],
                            wv_sb[:, a, :],
                            start=(a == 0), stop=(a == 7),
                        )
                    nc.vector.tensor_copy(
                        vT_sb[:, m, :].rearrange("p (h e) -> p h e", e=65)[:, :, 0:64],
                        ps[:].rearrange("p (h e) -> p h e", e=64),
                    )

            # ---- attention, one head PAIR at a time ----
            with (
                tc.tile_pool(name="exps", bufs=6) as exps_pool,
                tc.tile_pool(name="rb", bufs=3) as rb_pool,
                tc.tile_pool(name="rc", bufs=1) as rc_pool,
                tc.tile_pool(name="s_ps", bufs=1, space="PSUM") as s_ps,
                tc.tile_pool(name="o_ps", bufs=1, space="PSUM") as o_ps,
            ):
                for hp in range(4):
                    h0, h1 = 2 * hp, 2 * hp + 1
                    op0 = o_ps.tile([65, T], f32, tag="op0")
                    op1 = o_ps.tile([65, T], f32, tag="op1")
                    mega0 = exps_pool.tile([128, 8, T], bf16, tag="exps")
                    mega1 = exps_pool.tile([128, 8, T], bf16, tag="exps")
                    megas = (mega0, mega1)

                    def emit_o(it):
                        # O (rows 0..63) + colsum (row 64) on unnormalized exp
                        for g, (h, op) in enumerate(((h0, op0), (h1, op1))):
                            for jn in range(2):
                                nc.tensor.matmul(
                                    op[:, 512 * jn:512 * jn + 512],
                                    vT_sb[:, it, 65 * h:65 * h + 65],
                                    megas[g][:, it, 512 * jn:512 * jn + 512],
                                    start=(it == 0), stop=(it == 7),
                                )

                    for it in range(8):
                        sp0 = s_ps.tile([128, T], f32, tag="s0")
                        sp1 = s_ps.tile([128, T], f32, tag="s1")
                        # paired S matmuls on alternating row groups
                        for jn in range(2):
                            for base, sp in ((0, sp0), (64, sp1)):
                                nc.tensor.matmul(
                                    sp[:, 512 * jn:512 * jn + 512],
                                    k_sb[base:base + 64, hp, 128 * it:128 * it + 128],
                                    q_sb[base:base + 64, hp, 512 * jn:512 * jn + 512],
                                    start=True, stop=True,
                                )
                        nc.scalar.activation(
                            out=mega0[:, it, :], in_=sp0[:], func=Exp,
                            bias=kmask_sb[:, it:it + 1], scale=1.0)
                        nc.scalar.activation(
                            out=mega1[:, it, :], in_=sp1[:], func=Exp,
                            bias=kmask_sb[:, it:it + 1], scale=1.0)
                        # keep PE a tile behind ACT so it never waits on exp
                        if it >= 1:
                            emit_o(it - 1)
                    emit_o(7)
                    # factor = qmask / colsum -> bf16, broadcast to 128 partitions
                    # (approx recip mis-reads PSUM at base partition 64 — hop
                    # through SBUF via ACT first)
                    rbs = []
                    for g, op in ((0, op0), (1, op1)):
                        csr = rc_pool.tile([1, T], f32, tag="csr")
                        nc.scalar.copy(out=csr[:], in_=op[64:65, :])
                        rc = rc_pool.tile([1, T], f32, tag="rc")
                        nc.vector.reciprocal_approx_fast(out=rc[:], in_=csr[:])
                        rcb = rc_pool.tile([1, T], bf16, tag="rcb")
                        nc.vector.tensor_mul(rcb[:], rc[:], qmask_sb[:])
                        rb = rb_pool.tile([128, T], bf16, tag="rb")
                        nc.gpsimd.partition_broadcast(rb[:], rcb[0:1, :])
                        rbs.append(rb)
                    # normalize O during PSUM evac (includes qmask zeroing)
                    nc.vector.tensor_mul(
                        o_all[0:64, hp, :], op0[0:64, :], rbs[0][0:64, :])
                    nc.vector.tensor_mul(
                        stage_sb[:], op1[0:64, :], rbs[1][0:64, :])
                    nc.sync.dma_start(
                        out=o_all[64:128, hp, :], in_=stage_sb[:])
                    # normalize P in place on DVE (bf16 2x mode), write att
                    for it in range(8):
                        for g in (0, 1):
                            nc.vector.tensor_mul(
                                megas[g][:, it, :], megas[g][:, it, :],
                                rbs[g][:])
                    for g, h in ((0, h0), (1, h1)):
                        eng = nc.sync if g == 0 else nc.gpsimd
                        eng.dma_start(
                            out=att_d[h].rearrange("(it p) j -> p it j", p=128),
                            in_=megas[g][:])

            # ---- output projection (partial: host adds x and the peer core) ----
            with (
                tc.tile_pool(name="outw", bufs=1) as outw,
                tc.tile_pool(name="out_ps", bufs=2, space="PSUM") as out_ps,
            ):
                wo_sb = outw.tile([128, 4, T], f32r)
                nc.sync.dma_start(
                    out=wo_sb[:], in_=wo_d.rearrange("(a p) c -> p a c", p=128))
                for m in range(8):
                    ot = out_pool.tile([128, T], f32r, tag="ot")
                    for n in range(2):
                        ps = out_ps.tile([128, 512], f32, tag="ops")
                        for a in range(4):
                            nc.tensor.matmul(
                                ps[:],
                                wo_sb[:, a, 128 * m:128 * m + 128],
                                o_all[:, a, 512 * n:512 * n + 512],
                                start=(a == 0), stop=(a == 3),
                            )
                        nc.vector.tensor_copy(
                            ot[:, 512 * n:512 * n + 512], ps[:])
                    nc.scalar.dma_start(
                        out=out_d[128 * m:128 * m + 128, :], in_=ot[:])

    nc.compile()
    return nc


def _get_program():
    if "nc" not in _CACHE:
        _CACHE["nc"] = _build_program()
    return _CACHE["nc"]


def make_in_maps(x, mask, w_kvq, w_out):
    """Build the 8 per-core input maps (core c = batch c//2, head-group c%2)."""
    import ml_dtypes
    x = np.asarray(x, dtype=np.float32)
    mask = np.asarray(mask)
    w_kvq = np.asarray(w_kvq, dtype=np.float32)
    w_out = np.asarray(w_out, dtype=np.float32)

    w_k, w_v, w_q = w_kvq[0:C], w_kvq[C:2 * C], w_kvq[2 * C:3 * C]
    ones = np.ones((128, 1), dtype=ml_dtypes.bfloat16)
    in_maps = []
    for c in range(NCORES):
        bb, g = c // 2, c % 2
        rows = slice(512 * g, 512 * (g + 1))
        madd = np.where(mask[bb], np.float32(-1e30), np.float32(0.0)).astype(np.float32)
        in_maps.append({
            "x": np.ascontiguousarray(x[bb]),
            "wk": np.ascontiguousarray((w_k[rows] / 8.0).T.astype(np.float32)),
            "wq": np.ascontiguousarray(w_q[rows].T),
            "wv": np.ascontiguousarray(w_v[rows].T),
            "wo": np.ascontiguousarray(w_out[:, rows].T),
            "kmask": np.ascontiguousarray(madd.reshape(8, 128).T),
            "qmask": (~mask[bb]).astype(np.float32).reshape(1, T),
            "ones": ones,
        })
    return in_maps


def kernel(x, mask, w_kvq, w_out, trace=False):
    from concourse.bass_utils import run_bass_kernel_spmd

    nc = _get_program()
    in_maps = make_in_maps(x, mask, w_kvq, w_out)
    res = run_bass_kernel_spmd(nc, in_maps, core_ids=list(range(NCORES)),
                               trace=trace)

    out_full = np.empty((B, C, T), dtype=np.float32)
    att_flat = np.empty((H * B, T, T), dtype=np.float32)
    att_view = att_flat.reshape(H, B, T, T)
    x = np.asarray(x, dtype=np.float32)
    for c in range(NCORES):
        bb, g = c // 2, c % 2
        att_view[HPC * g:HPC * (g + 1), bb] = res.results[c]["att"].astype(np.float32)
        if g == 0:
            out_full[bb] = x[bb] + res.results[c]["out"]
        else:
            out_full[bb] += res.results[c]["out"]
    if trace:
        _CACHE["last_exec_time_ns"] = res.exec_time_ns
    return out_full, att_flat


# revision 20
# speedup vs baseline: 1.0138x; 1.0138x over previous
"""Multi-head attention (nn_MultiHeadAttention) Trainium2 Bass kernel.

Sharding: 8 cores = 4 batches x 2 head-groups (8 heads each).
Per core, heads are processed in pairs living on partition rows 0-63 /
64-127 so the K=64 S-matmuls alternate PE row-groups (concurrent sub-array
execution). The key mask folds into the exp bias; O and the softmax
denominator come from one M=65 matmul (ones column in vT) over the
unnormalized exp, so PE never waits on the normalize chain. The exp tiles,
vT, and the normalize factor are bf16: DVE runs the in-place normalize in
2x mode and the att DMA moves half the bytes. S scores and the k/q/o/w
tensors stay float32r. Host adds x + the two head-group partials.
"""
import sys
sys.path.insert(0, "/opt/trn_rl_repo")

import numpy as np

B, C, T, H = 4, 1024, 1024, 16
D = C // H            # 64 head dim
HPC = 8               # heads per core
NCORES = 8

_CACHE = {}


def _build_program():
    import concourse.bass as bass
    import concourse.mybir as mybir
    import concourse.tile as tile
    from concourse import bacc

    f32 = mybir.dt.float32
    f32r = mybir.dt.float32r
    bf16 = mybir.dt.bfloat16

    nc = bacc.Bacc("TRN2", target_bir_lowering=False, debug=False,
                   num_devices=NCORES)

    x_d = nc.dram_tensor("x", [C, T], f32r, kind="ExternalInput").ap()
    wk_d = nc.dram_tensor("wk", [C, 512], f32r, kind="ExternalInput").ap()
    wq_d = nc.dram_tensor("wq", [C, 512], f32r, kind="ExternalInput").ap()
    wv_d = nc.dram_tensor("wv", [C, 512], f32r, kind="ExternalInput").ap()
    wo_d = nc.dram_tensor("wo", [512, C], f32r, kind="ExternalInput").ap()
    kmask_d = nc.dram_tensor("kmask", [128, 8], f32, kind="ExternalInput").ap()
    qmask_d = nc.dram_tensor("qmask", [1, T], f32, kind="ExternalInput").ap()
    ones_d = nc.dram_tensor("ones", [128, 1], bf16, kind="ExternalInput").ap()

    att_d = nc.dram_tensor("att", [HPC, T, T], bf16, kind="ExternalOutput").ap()
    out_d = nc.dram_tensor("out", [C, T], f32r, kind="ExternalOutput").ap()

    Exp = mybir.ActivationFunctionType.Exp

    with tile.TileContext(nc) as tc:
        with (
            tc.tile_pool(name="persist", bufs=1) as persist,
            tc.tile_pool(name="out_sb", bufs=2) as out_pool,
        ):
            k_sb = persist.tile([128, 4, T], f32r)
            q_sb = persist.tile([128, 4, T], f32r)
            # per head: 64 v columns + a ones column => 65 wide (bf16)
            vT_sb = persist.tile([128, 8, 8 * 65], bf16)
            o_all = persist.tile([128, 4, T], f32r)
            wo_sb = persist.tile([128, 4, T], f32r)
            kmask_sb = persist.tile([128, 8], f32)
            qmask_sb = persist.tile([1, T], f32)
            stage_sb = persist.tile([64, T], f32r)

            nc.scalar.dma_start(out=kmask_sb[:], in_=kmask_d[:])
            nc.scalar.dma_start(
                out=wo_sb[:], in_=wo_d.rearrange("(a p) c -> p a c", p=128))
            nc.gpsimd.dma_start(out=qmask_sb[:], in_=qmask_d[:])
            # ones columns of vT (one per head, column 65h+64): memset a ones
            # tile and fan it out with one strided DVE copy (a DMA here would
            # emit thousands of 2-byte descriptors)
            ones_sb = persist.tile([128, 1], bf16)
            nc.vector.memset(ones_sb[:], 1.0)
            ones_rep = bass.AP(
                tensor=ones_sb[:].tensor, offset=ones_sb[:].offset,
                ap=[list(p) for p in ones_sb[:].ap[:1]] + [[0, 8], [0, 8], [0, 1]])
            nc.vector.tensor_copy(
                vT_sb[:].rearrange("p m (h e) -> p m h e", e=65)[:, :, :, 64:65],
                ones_rep)

            # ---- projections ----
            with (
                tc.tile_pool(name="projin", bufs=1) as projin,
                tc.tile_pool(name="proj_ps", bufs=4, space="PSUM") as proj_ps,
            ):
                x_sb = projin.tile([128, 8, T], f32r)
                wk_sb = projin.tile([128, 8, 512], f32r)
                wq_sb = projin.tile([128, 8, 512], f32r)
                wv_sb = projin.tile([128, 8, 512], f32r)
                # one big DMA per tensor (contiguous dst = full ring rate),
                # spread across the three DGE queues
                nc.sync.dma_start(
                    out=x_sb[:], in_=x_d.rearrange("(a p) t -> p a t", p=128))
                nc.scalar.dma_start(
                    out=wk_sb[:], in_=wk_d.rearrange("(a p) m -> p a m", p=128))
                nc.gpsimd.dma_start(
                    out=wq_sb[:], in_=wq_d.rearrange("(a p) m -> p a m", p=128))
                nc.scalar.dma_start(
                    out=wv_sb[:], in_=wv_d.rearrange("(a p) m -> p a m", p=128))

                # k, q projections: out rows m-tile (128 rows = head pair)
                for m in range(4):
                    for w_sb, dst in ((wk_sb, k_sb), (wq_sb, q_sb)):
                        for n in range(2):
                            ps = proj_ps.tile([128, 512], f32, tag="pp")
                            for a in range(8):
                                nc.tensor.matmul(
                                    ps[:],
                                    w_sb[:, a, 128 * m:128 * m + 128],
                                    x_sb[:, a, 512 * n:512 * n + 512],
                                    start=(a == 0), stop=(a == 7),
                                )
                            nc.vector.tensor_copy(
                                dst[:, m, 512 * n:512 * n + 512], ps[:])
                # vT projection: out [t-tile, 512 cv] -> strided per-head 65 layout
                for m in range(8):
                    ps = proj_ps.tile([128, 512], f32, tag="pp")
                    for a in range(8):
                        nc.tensor.matmul(
                            ps[:],
                            x_sb[:, a, 128 * m:128 * m + 128],
                            wv_sb[:, a, :],
                            start=(a == 0), stop=(a == 7),
                        )
                    nc.vector.tensor_copy(
                        vT_sb[:, m, :].rearrange("p (h e) -> p h e", e=65)[:, :, 0:64],
                        ps[:].rearrange("p (h e) -> p h e", e=64),
                    )

            # ---- attention, one head PAIR at a time ----
            with (
                tc.tile_pool(name="exps", bufs=5) as exps_pool,
                tc.tile_pool(name="rb", bufs=2) as rb_pool,
                tc.tile_pool(name="rc", bufs=1) as rc_pool,
                tc.tile_pool(name="s_ps", bufs=1, space="PSUM") as s_ps,
                tc.tile_pool(name="o_ps", bufs=1, space="PSUM") as o_ps,
            ):
                for hp in range(4):
                    h0, h1 = 2 * hp, 2 * hp + 1
                    op0 = o_ps.tile([65, T], f32, tag="op0")
                    op1 = o_ps.tile([65, T], f32, tag="op1")
                    mega0 = exps_pool.tile([128, 8, T], bf16, tag="exps")
                    mega1 = exps_pool.tile([128, 8, T], bf16, tag="exps")
                    megas = (mega0, mega1)

                    def emit_o(it):
                        # O (rows 0..63) + colsum (row 64) on unnormalized exp
                        for g, (h, op) in enumerate(((h0, op0), (h1, op1))):
                            for jn in range(2):
                                nc.tensor.matmul(
                                    op[:, 512 * jn:512 * jn + 512],
                                    vT_sb[:, it, 65 * h:65 * h + 65],
                                    megas[g][:, it, 512 * jn:512 * jn + 512],
                                    start=(it == 0), stop=(it == 7),
                                )

                    for it in range(8):
                        sp0 = s_ps.tile([128, T], f32, tag="s0")
                        sp1 = s_ps.tile([128, T], f32, tag="s1")
                        # paired S matmuls on alternating row groups
                        for jn in range(2):
                            for base, sp in ((0, sp0), (64, sp1)):
                                nc.tensor.matmul(
                                    sp[:, 512 * jn:512 * jn + 512],
                                    k_sb[base:base + 64, hp, 128 * it:128 * it + 128],
                                    q_sb[base:base + 64, hp, 512 * jn:512 * jn + 512],
                                    start=True, stop=True,
                                )
                        nc.scalar.activation(
                            out=mega0[:, it, :], in_=sp0[:], func=Exp,
                            bias=kmask_sb[:, it:it + 1], scale=1.0)
                        nc.scalar.activation(
                            out=mega1[:, it, :], in_=sp1[:], func=Exp,
                            bias=kmask_sb[:, it:it + 1], scale=1.0)
                        # keep PE a tile behind ACT so it never waits on exp
                        if it >= 1:
                            emit_o(it - 1)
                    emit_o(7)
                    # factor = qmask / colsum -> bf16, broadcast to 128 partitions
                    # (approx recip mis-reads PSUM at base partition 64 — hop
                    # through SBUF via ACT first)
                    rbs = []
                    for g, op in ((0, op0), (1, op1)):
                        csr = rc_pool.tile([1, T], f32, tag="csr")
                        nc.scalar.copy(out=csr[:], in_=op[64:65, :])
                        rc = rc_pool.tile([1, T], f32, tag="rc")
                        nc.vector.reciprocal_approx_fast(out=rc[:], in_=csr[:])
                        rcb = rc_pool.tile([1, T], bf16, tag="rcb")
                        nc.vector.tensor_mul(rcb[:], rc[:], qmask_sb[:])
                        rb = rb_pool.tile([128, T], bf16, tag="rb")
                        nc.gpsimd.partition_broadcast(rb[:], rcb[0:1, :])
                        rbs.append(rb)
                    # normalize O during PSUM evac (includes qmask zeroing)
                    nc.vector.tensor_mul(
                        o_all[0:64, hp, :], op0[0:64, :], rbs[0][0:64, :])
                    nc.vector.tensor_mul(
                        stage_sb[:], op1[0:64, :], rbs[1][0:64, :])
                    nc.sync.dma_start(
                        out=o_all[64:128, hp, :], in_=stage_sb[:])
                    # normalize P in place on DVE (bf16 2x mode), write att
                    for it in range(8):
                        for g in (0, 1):
                            nc.vector.tensor_mul(
                                megas[g][:, it, :], megas[g][:, it, :],
                                rbs[g][:])
                    for g, h in ((0, h0), (1, h1)):
                        eng = nc.sync if g == 0 else nc.gpsimd
                        eng.dma_start(
                            out=att_d[h].rearrange("(it p) j -> p it j", p=128),
                            in_=megas[g][:])

            # ---- output projection (partial: host adds x and the peer core) ----
            with (
                tc.tile_pool(name="out_ps", bufs=2, space="PSUM") as out_ps,
            ):
                for m in range(8):
                    ot = out_pool.tile([128, T], f32r, tag="ot")
                    for n in range(2):
                        ps = out_ps.tile([128, 512], f32, tag="ops")
                        for a in range(4):
                            nc.tensor.matmul(
                                ps[:],
                                wo_sb[:, a, 128 * m:128 * m + 128],
                                o_all[:, a, 512 * n:512 * n + 512],
                                start=(a == 0), stop=(a == 3),
                            )
                        nc.vector.tensor_copy(
                            ot[:, 512 * n:512 * n + 512], ps[:])
                    nc.scalar.dma_start(
                        out=out_d[128 * m:128 * m + 128, :], in_=ot[:])

    nc.compile()
    return nc


def _get_program():
    if "nc" not in _CACHE:
        _CACHE["nc"] = _build_program()
    return _CACHE["nc"]


def make_in_maps(x, mask, w_kvq, w_out):
    """Build the 8 per-core input maps (core c = batch c//2, head-group c%2)."""
    import ml_dtypes
    x = np.asarray(x, dtype=np.float32)
    mask = np.asarray(mask)
    w_kvq = np.asarray(w_kvq, dtype=np.float32)
    w_out = np.asarray(w_out, dtype=np.float32)

    w_k, w_v, w_q = w_kvq[0:C], w_kvq[C:2 * C], w_kvq[2 * C:3 * C]
    ones = np.ones((128, 1), dtype=ml_dtypes.bfloat16)
    in_maps = []
    for c in range(NCORES):
        bb, g = c // 2, c % 2
        rows = slice(512 * g, 512 * (g + 1))
        madd = np.where(mask[bb], np.float32(-1e30), np.float32(0.0)).astype(np.float32)
        in_maps.append({
            "x": np.ascontiguousarray(x[bb]),
            "wk": np.ascontiguousarray((w_k[rows] / 8.0).T.astype(np.float32)),
            "wq": np.ascontiguousarray(w_q[rows].T),
            "wv": np.ascontiguousarray(w_v[rows].T),
            "wo": np.ascontiguousarray(w_out[:, rows].T),
            "kmask": np.ascontiguousarray(madd.reshape(8, 128).T),
            "qmask": (~mask[bb]).astype(np.float32).reshape(1, T),
            "ones": ones,
        })
    return in_maps


def kernel(x, mask, w_kvq, w_out, trace=False):
    from concourse.bass_utils import run_bass_kernel_spmd

    nc = _get_program()
    in_maps = make_in_maps(x, mask, w_kvq, w_out)
    res = run_bass_kernel_spmd(nc, in_maps, core_ids=list(range(NCORES)),
                               trace=trace)

    out_full = np.empty((B, C, T), dtype=np.float32)
    att_flat = np.empty((H * B, T, T), dtype=np.float32)
    att_view = att_flat.reshape(H, B, T, T)
    x = np.asarray(x, dtype=np.float32)
    for c in range(NCORES):
        bb, g = c // 2, c % 2
        att_view[HPC * g:HPC * (g + 1), bb] = res.results[c]["att"].astype(np.float32)
        if g == 0:
            out_full[bb] = x[bb] + res.results[c]["out"]
        else:
            out_full[bb] += res.results[c]["out"]
    if trace:
        _CACHE["last_exec_time_ns"] = res.exec_time_ns
    return out_full, att_flat


# revision 21
# speedup vs baseline: 1.1330x; 1.1175x over previous
"""Multi-head attention (nn_MultiHeadAttention) Trainium2 Bass kernel.

Sharding: 8 cores = 4 batches x 2 head-groups (8 heads each).
Per core, heads are processed in pairs living on partition rows 0-63 /
64-127 so the K=64 S-matmuls alternate PE row-groups (concurrent sub-array
execution). The key mask folds into the exp bias; O and the softmax
denominator come from one M=65 matmul (ones column in vT) over the
unnormalized exp, so PE never waits on the normalize chain. The exp tiles,
vT, and the normalize factor are bf16: DVE runs the in-place normalize in
2x mode and the att DMA moves half the bytes. S scores and the k/q/o/w
tensors stay float32r. Host adds x + the two head-group partials.
"""
import sys
sys.path.insert(0, "/opt/trn_rl_repo")

import numpy as np

B, C, T, H = 4, 1024, 1024, 16
D = C // H            # 64 head dim
HPC = 8               # heads per core
NCORES = 8

_CACHE = {}


def _build_program():
    import concourse.bass as bass
    import concourse.mybir as mybir
    import concourse.tile as tile
    from concourse import bacc

    f32 = mybir.dt.float32
    f32r = mybir.dt.float32r
    bf16 = mybir.dt.bfloat16

    nc = bacc.Bacc("TRN2", target_bir_lowering=False, debug=False,
                   num_devices=NCORES)

    x_d = nc.dram_tensor("x", [C, T], f32r, kind="ExternalInput").ap()
    wk_d = nc.dram_tensor("wk", [C, 512], f32r, kind="ExternalInput").ap()
    wq_d = nc.dram_tensor("wq", [C, 512], f32r, kind="ExternalInput").ap()
    wv_d = nc.dram_tensor("wv", [C, 512], f32r, kind="ExternalInput").ap()
    wo_d = nc.dram_tensor("wo", [512, C], f32r, kind="ExternalInput").ap()
    kmask_d = nc.dram_tensor("kmask", [128, 8], f32, kind="ExternalInput").ap()
    qmask_d = nc.dram_tensor("qmask", [1, T], f32, kind="ExternalInput").ap()
    ones_d = nc.dram_tensor("ones", [128, 1], bf16, kind="ExternalInput").ap()

    att_d = nc.dram_tensor("att", [HPC, T, T], bf16, kind="ExternalOutput").ap()
    out_d = nc.dram_tensor("out", [C, T], f32r, kind="ExternalOutput").ap()

    Exp = mybir.ActivationFunctionType.Exp

    with tile.TileContext(nc) as tc:
        with (
            tc.tile_pool(name="persist", bufs=1) as persist,
            tc.tile_pool(name="out_sb", bufs=2) as out_pool,
        ):
            k_sb = persist.tile([128, 4, T], f32r)
            q_sb = persist.tile([128, 4, T], f32r)
            # per head: 64 v columns + a ones column => 65 wide (bf16)
            vT_sb = persist.tile([128, 8, 8 * 65], bf16)
            o_all = persist.tile([128, 4, T], f32r)
            wo_sb = persist.tile([128, 4, T], f32r)
            kmask_sb = persist.tile([128, 8], f32)
            qmask_sb = persist.tile([1, T], f32)
            stage_sb = persist.tile([64, T], f32r)

            nc.scalar.dma_start(out=kmask_sb[:], in_=kmask_d[:])
            nc.scalar.dma_start(
                out=wo_sb[:], in_=wo_d.rearrange("(a p) c -> p a c", p=128))
            nc.gpsimd.dma_start(out=qmask_sb[:], in_=qmask_d[:])
            # ones columns of vT (one per head, column 65h+64): memset a ones
            # tile and fan it out with one strided DVE copy (a DMA here would
            # emit thousands of 2-byte descriptors)
            ones_sb = persist.tile([128, 1], bf16)
            nc.vector.memset(ones_sb[:], 1.0)
            ones_rep = bass.AP(
                tensor=ones_sb[:].tensor, offset=ones_sb[:].offset,
                ap=[list(p) for p in ones_sb[:].ap[:1]] + [[0, 8], [0, 8], [0, 1]])
            nc.vector.tensor_copy(
                vT_sb[:].rearrange("p m (h e) -> p m h e", e=65)[:, :, :, 64:65],
                ones_rep)

            # ---- projections ----
            with (
                tc.tile_pool(name="projin", bufs=1) as projin,
                tc.tile_pool(name="proj_ps", bufs=4, space="PSUM") as proj_ps,
            ):
                x_sb = projin.tile([128, 8, T], f32r)
                wk_sb = projin.tile([128, 8, 512], f32r)
                wq_sb = projin.tile([128, 8, 512], f32r)
                wv_sb = projin.tile([128, 8, 512], f32r)
                # one big DMA per tensor (contiguous dst = full ring rate),
                # spread across the three DGE queues
                nc.sync.dma_start(
                    out=x_sb[:], in_=x_d.rearrange("(a p) t -> p a t", p=128))
                nc.scalar.dma_start(
                    out=wk_sb[:], in_=wk_d.rearrange("(a p) m -> p a m", p=128))
                nc.gpsimd.dma_start(
                    out=wq_sb[:], in_=wq_d.rearrange("(a p) m -> p a m", p=128))
                nc.scalar.dma_start(
                    out=wv_sb[:], in_=wv_d.rearrange("(a p) m -> p a m", p=128))

                # k, q projections: out rows m-tile (128 rows = head pair)
                for m in range(4):
                    for w_sb, dst in ((wk_sb, k_sb), (wq_sb, q_sb)):
                        for n in range(2):
                            ps = proj_ps.tile([128, 512], f32, tag="pp")
                            for a in range(8):
                                nc.tensor.matmul(
                                    ps[:],
                                    w_sb[:, a, 128 * m:128 * m + 128],
                                    x_sb[:, a, 512 * n:512 * n + 512],
                                    start=(a == 0), stop=(a == 7),
                                )
                            nc.vector.tensor_copy(
                                dst[:, m, 512 * n:512 * n + 512], ps[:])
                # vT projection: out [t-tile, 512 cv] -> strided per-head 65 layout
                for m in range(8):
                    ps = proj_ps.tile([128, 512], f32, tag="pp")
                    for a in range(8):
                        nc.tensor.matmul(
                            ps[:],
                            x_sb[:, a, 128 * m:128 * m + 128],
                            wv_sb[:, a, :],
                            start=(a == 0), stop=(a == 7),
                        )
                    nc.vector.tensor_copy(
                        vT_sb[:, m, :].rearrange("p (h e) -> p h e", e=65)[:, :, 0:64],
                        ps[:].rearrange("p (h e) -> p h e", e=64),
                    )

            # ---- attention, one head PAIR at a time ----
            with (
                tc.tile_pool(name="exps", bufs=6) as exps_pool,
                tc.tile_pool(name="rb", bufs=2) as rb_pool,
                tc.tile_pool(name="rc", bufs=1) as rc_pool,
                tc.tile_pool(name="s_ps", bufs=1, space="PSUM") as s_ps,
                tc.tile_pool(name="o_ps", bufs=1, space="PSUM") as o_ps,
            ):
                for hp in range(4):
                    h0, h1 = 2 * hp, 2 * hp + 1
                    op0 = o_ps.tile([65, T], f32, tag="op0")
                    op1 = o_ps.tile([65, T], f32, tag="op1")
                    mega0 = exps_pool.tile([128, 8, T], bf16, tag="exps")
                    mega1 = exps_pool.tile([128, 8, T], bf16, tag="exps")
                    megas = (mega0, mega1)

                    def emit_o(it):
                        # O (rows 0..63) + colsum (row 64) on unnormalized exp
                        for g, (h, op) in enumerate(((h0, op0), (h1, op1))):
                            for jn in range(2):
                                nc.tensor.matmul(
                                    op[:, 512 * jn:512 * jn + 512],
                                    vT_sb[:, it, 65 * h:65 * h + 65],
                                    megas[g][:, it, 512 * jn:512 * jn + 512],
                                    start=(it == 0), stop=(it == 7),
                                )

                    for it in range(8):
                        sp0 = s_ps.tile([128, T], f32, tag="s0")
                        sp1 = s_ps.tile([128, T], f32, tag="s1")
                        # paired S matmuls on alternating row groups
                        for jn in range(2):
                            for base, sp in ((0, sp0), (64, sp1)):
                                nc.tensor.matmul(
                                    sp[:, 512 * jn:512 * jn + 512],
                                    k_sb[base:base + 64, hp, 128 * it:128 * it + 128],
                                    q_sb[base:base + 64, hp, 512 * jn:512 * jn + 512],
                                    start=True, stop=True,
                                )
                        nc.scalar.activation(
                            out=mega0[:, it, :], in_=sp0[:], func=Exp,
                            bias=kmask_sb[:, it:it + 1], scale=1.0)
                        nc.scalar.activation(
                            out=mega1[:, it, :], in_=sp1[:], func=Exp,
                            bias=kmask_sb[:, it:it + 1], scale=1.0)
                        # keep PE a tile behind ACT so it never waits on exp
                        if it >= 1:
                            emit_o(it - 1)
                    emit_o(7)
                    # factor = qmask / colsum -> bf16, broadcast to 128 partitions
                    # (approx recip mis-reads PSUM at base partition 64 — hop
                    # through SBUF via ACT first)
                    rbs = []
                    for g, op in ((0, op0), (1, op1)):
                        csr = rc_pool.tile([1, T], f32, tag="csr")
                        nc.scalar.copy(out=csr[:], in_=op[64:65, :])
                        rc = rc_pool.tile([1, T], f32, tag="rc")
                        nc.vector.reciprocal_approx_fast(out=rc[:], in_=csr[:])
                        rcb = rc_pool.tile([1, T], bf16, tag="rcb")
                        nc.vector.tensor_mul(rcb[:], rc[:], qmask_sb[:])
                        rb = rb_pool.tile([128, T], bf16, tag="rb")
                        nc.gpsimd.partition_broadcast(rb[:], rcb[0:1, :])
                        rbs.append(rb)
                    # normalize O during PSUM evac (includes qmask zeroing)
                    nc.vector.tensor_mul(
                        o_all[0:64, hp, :], op0[0:64, :], rbs[0][0:64, :])
                    nc.vector.tensor_mul(
                        stage_sb[:], op1[0:64, :], rbs[1][0:64, :])
                    nc.sync.dma_start(
                        out=o_all[64:128, hp, :], in_=stage_sb[:])
                    # normalize P in place on DVE (bf16 2x mode), write att
                    for it in range(8):
                        for g in (0, 1):
                            nc.vector.tensor_mul(
                                megas[g][:, it, :], megas[g][:, it, :],
                                rbs[g][:])
                    for g, h in ((0, h0), (1, h1)):
                        eng = nc.sync if g == 0 else nc.gpsimd
                        eng.dma_start(
                            out=att_d[h].rearrange("(it p) j -> p it j", p=128),
                            in_=megas[g][:])

            # ---- output projection (partial: host adds x and the peer core) ----
            with (
                tc.tile_pool(name="out_ps", bufs=2, space="PSUM") as out_ps,
            ):
                for m in range(8):
                    ot = out_pool.tile([128, T], f32r, tag="ot")
                    for n in range(2):
                        ps = out_ps.tile([128, 512], f32, tag="ops")
                        for a in range(4):
                            nc.tensor.matmul(
                                ps[:],
                                wo_sb[:, a, 128 * m:128 * m + 128],
                                o_all[:, a, 512 * n:512 * n + 512],
                                start=(a == 0), stop=(a == 3),
                            )
                        nc.scalar.copy(
                            out=ot[:, 512 * n:512 * n + 512], in_=ps[:])
                    nc.scalar.dma_start(
                        out=out_d[128 * m:128 * m + 128, :], in_=ot[:])

    nc.compile()
    return nc


def _get_program():
    if "nc" not in _CACHE:
        _CACHE["nc"] = _build_program()
    return _CACHE["nc"]


def make_in_maps(x, mask, w_kvq, w_out):
    """Build the 8 per-core input maps (core c = batch c//2, head-group c%2)."""
    import ml_dtypes
    x = np.asarray(x, dtype=np.float32)
    mask = np.asarray(mask)
    w_kvq = np.asarray(w_kvq, dtype=np.float32)
    w_out = np.asarray(w_out, dtype=np.float32)

    w_k, w_v, w_q = w_kvq[0:C], w_kvq[C:2 * C], w_kvq[2 * C:3 * C]
    ones = np.ones((128, 1), dtype=ml_dtypes.bfloat16)
    in_maps = []
    for c in range(NCORES):
        bb, g = c // 2, c % 2
        rows = slice(512 * g, 512 * (g + 1))
        madd = np.where(mask[bb], np.float32(-1e30), np.float32(0.0)).astype(np.float32)
        in_maps.append({
            "x": np.ascontiguousarray(x[bb]),
            "wk": np.ascontiguousarray((w_k[rows] / 8.0).T.astype(np.float32)),
            "wq": np.ascontiguousarray(w_q[rows].T),
            "wv": np.ascontiguousarray(w_v[rows].T),
            "wo": np.ascontiguousarray(w_out[:, rows].T),
            "kmask": np.ascontiguousarray(madd.reshape(8, 128).T),
            "qmask": (~mask[bb]).astype(np.float32).reshape(1, T),
            "ones": ones,
        })
    return in_maps


def kernel(x, mask, w_kvq, w_out, trace=False):
    from concourse.bass_utils import run_bass_kernel_spmd

    nc = _get_program()
    in_maps = make_in_maps(x, mask, w_kvq, w_out)
    res = run_bass_kernel_spmd(nc, in_maps, core_ids=list(range(NCORES)),
                               trace=trace)

    out_full = np.empty((B, C, T), dtype=np.float32)
    att_flat = np.empty((H * B, T, T), dtype=np.float32)
    att_view = att_flat.reshape(H, B, T, T)
    x = np.asarray(x, dtype=np.float32)
    for c in range(NCORES):
        bb, g = c // 2, c % 2
        att_view[HPC * g:HPC * (g + 1), bb] = res.results[c]["att"].astype(np.float32)
        if g == 0:
            out_full[bb] = x[bb] + res.results[c]["out"]
        else:
            out_full[bb] += res.results[c]["out"]
    if trace:
        _CACHE["last_exec_time_ns"] = res.exec_time_ns
    return out_full, att_flat
